# revision 1
# baseline (speedup 1.0000x reference)
r"""DbrxAttention on 8 TRN2 NeuronCores, tensor-parallel across heads.

Per-core shard (core c of 8): 6 query heads (q heads 6c..6c+5), kv head c
(replicated per its 6-head query group), plus the matching 768 input
columns of the out-projection. Each core computes a partial out-proj
(row-parallel Wout); the partials are summed on the host (the all-reduce
of the TP pattern).

Layouts (per core, all device tensors):
  hidT   [6144, 2048] fp16  hidden^T       (d on partitions)
  wqkvT  [6144, 1024] fp16  [q0..q5 | k | v] columns of Wqkv^T shard
  woutT  [768,  6144] fp16  Wout[:, shard]^T
  cos/sin tables [128, 2048] fp16, neox rope with sign-folded sin and the
  1/sqrt(128) score scale folded into the q tables.
  masks  [128, 128] fp16  multiplicative causal mask (f >= p) for the
         in-tile triangle of diagonal score tiles

Structure (PE runs ~99% busy in the schedule sim):
- Each QKV sweep (512-t chunk) is two passes over the 48 d-chunks reading
  a persistent SBUF hid slab: pass A computes q0..q5 in three 2-bank
  "wide" PSUM tiles; pass B computes k + v in the 2-bank ring. During
  pass B the wides are free, so the previous chunk's attention chains
  interleave into the in-order PE stream (their exp/DVE latency hides
  under k/v GEMM work).
- Chains (per head, 512-q chunk): kt-pair score matmuls into one wide
  PSUM tile -> single 1024-wide exp on ACT -> bf16 probs; diagonal kt
  tiles compute only the causal q-suffix (15% less score/v work) with a
  single 128x128 triangle mask on DVE; softmax row sums accumulate on
  DVE in bf16 legs + one POOL partition_all_reduce per chain (fp32
  internal, result broadcast to all partitions for free); attention
  accumulates in the "attnw" wide PSUM tile; normalization is all-bf16
  (ACT copy, DVE reciprocal, DVE multiply).
- The last chunk's chains run after QKV with every out-proj group's
  (oc, tt) blocks rate-interleaved into the PE stream; out-proj PSUM
  evacuations alternate ACT/DVE (GPSIMD cannot read PSUM) and partial
  fp16 outputs stream per t-tile. Host sums the 8 partials in fp32.
- Queue routing keeps the in-order DMA queues unblocked: hid slab on
  ACT, weights on SP, rope partition-swaps + tables on POOL; the last
  sweep's k-rope runs on DVE so the final chains aren't queued behind
  the q-ropes on POOL.
"""

import os

import numpy as np

import concourse.mybir as mybir
import concourse.tile as tile
from concourse import bacc
from concourse import bass_isa
from concourse.bass_utils import run_bass_kernel_spmd

F32R = mybir.dt.float32r
F32 = mybir.dt.float32
F16 = mybir.dt.float16
BF16 = mybir.dt.bfloat16

T = 2048
D = 6144
N_HEADS = 48
N_KV = 8
HD = 128
CLIP = 8.0
THETA = 500000.0
N_CORES = 8
HPC = N_HEADS // N_CORES      # q heads per core = 6
QKJ = HPC + 1                 # q+k j-tiles per core = 7
DCH = D // 128                # 48 contraction chunks
DG = DCH // 4                 # 12 batched (4-chunk) DMA groups
TCH = T // 512                # 4 t-chunks
TTILES = T // 128             # 16 t-tiles
OCH = D // 512                # 12 out-proj column chunks
ICH = HPC                     # 6 out-proj contraction chunks (768/128)

_compiled = None


def _build():
    nc = bacc.Bacc("TRN2", target_bir_lowering=False, debug=False,
                   num_devices=N_CORES)

    hidT_d = nc.dram_tensor("hidT", [D, T], F16, kind="ExternalInput").ap()
    wqkvT_d = nc.dram_tensor("wqkvT", [D, 1024], F16, kind="ExternalInput").ap()
    woutT_d = nc.dram_tensor("woutT", [HPC * HD, D], F16, kind="ExternalInput").ap()
    cosq_d = nc.dram_tensor("cosq", [HD, T], F16, kind="ExternalInput").ap()
    sinq_d = nc.dram_tensor("sinq", [HD, T], F16, kind="ExternalInput").ap()
    cosk_d = nc.dram_tensor("cosk", [HD, T], F16, kind="ExternalInput").ap()
    sink_d = nc.dram_tensor("sink", [HD, T], F16, kind="ExternalInput").ap()
    mask_d = nc.dram_tensor("maskm", [HD, 128], F16, kind="ExternalInput").ap()
    outp_d = nc.dram_tensor("outp", [T, D], F16, kind="ExternalOutput").ap()

    mn, mx = mybir.AluOpType.min, mybir.AluOpType.max
    mult, add = mybir.AluOpType.mult, mybir.AluOpType.add
    EXP = mybir.ActivationFunctionType.Exp

    with tile.TileContext(nc) as tc:
        with (
            tc.tile_pool(name="sb", bufs=1) as pool,
            tc.tile_pool(name="ps", bufs=1, space="PSUM") as psum,
        ):
            # persistent tensors
            qkT = pool.tile([128, QKJ, T], F16)       # roped q (scaled) + k
            v_sb = pool.tile([128, TTILES, HD], BF16)  # clipped v, [t%128, t//128, hd]
            attnT = pool.tile([128, HPC, T], F16)      # normalized attn^T
            cosq = pool.tile([HD, T], F16)
            sinq = pool.tile([HD, T], F16)
            cosk = pool.tile([HD, T], F16)
            sink = pool.tile([HD, T], F16)
            masks = pool.tile([HD, 128], F16)
            # persistent hid slab: per-sweep writes overwrite slices, so the
            # WAR against the previous sweep's readers is tracked per-slice
            # (a per-sweep pool.tile would bump the whole-tile version and
            # serialize the refill behind all of pass B)
            hslab = pool.tile([128, DCH, 512], F16)

            def load_tables():
                nc.gpsimd.dma_start(cosq[:], cosq_d[:])
                nc.gpsimd.dma_start(sinq[:], sinq_d[:])
                nc.gpsimd.dma_start(cosk[:], cosk_d[:])
                nc.gpsimd.dma_start(sink[:], sink_d[:])
                nc.gpsimd.dma_start(masks[:], mask_d[:])

            def qkv_sweep(tcx, interleave=None):
                # Pass A: q heads j0..j5 accumulate in three 2-bank "wide"
                # PSUM tiles (2 from the sc2 ring + the attnw tile) over all
                # 48 d-chunks; hid lands in a resident SBUF slab. Pass B:
                # k (j6) and v accumulate in the 2-bank ring re-reading the
                # slab (no second hid DMA). During pass B the wides are free
                # again, so the previous chunk's attention chains interleave
                # into the PE stream here (hiding the chain latency that
                # otherwise pays off only after the last sweep).
                tsl = slice(tcx * 512, (tcx + 1) * 512)
                widesA = [psum.tile([128, 1024], F32, tag="wide", bufs=2,
                                    name=f"qkw{w}") for w in range(2)]
                widesA.append(psum.tile([128, 1024], F32, tag="attnw", bufs=1,
                                        name="qkw2"))
                qk_ps = [widesA[j // 2][:, (j % 2) * 512:(j % 2 + 1) * 512]
                         for j in range(6)]
                for g in range(DG):
                    g4 = slice(g * 512, (g + 1) * 512)
                    wqa = pool.tile([128, 4, 768], F16, tag="wqa", bufs=3)
                    if tcx == 0 and g == 0:
                        for i in range(4):
                            dsl = slice(i * 128, (i + 1) * 128)
                            nc.sync.dma_start(wqa[:, i, :],
                                              wqkvT_d[dsl, 0:768])
                            nc.scalar.dma_start(hslab[:, i, :],
                                                hidT_d[dsl, tsl])
                    else:
                        nc.scalar.dma_start(
                            hslab[:, g * 4:(g + 1) * 4, :],
                            hidT_d[g4, tsl].rearrange("(a p) t -> p a t",
                                                      p=128))
                        nc.sync.dma_start(
                            wqa[:], wqkvT_d[g4, 0:768].rearrange(
                                "(a p) w -> p a w", p=128))
                    for i in range(4):
                        d = g * 4 + i
                        st, sp = d == 0, d == DCH - 1
                        for j in range(6):
                            nc.tensor.matmul(qk_ps[j],
                                             wqa[:, i, j * 128:(j + 1) * 128],
                                             hslab[:, d, :], start=st,
                                             stop=sp, skip_group_check=True)
                # evac A: clips first (release the wides for the interleaved
                # chains), then ropes for q0..q5 on POOL
                rawsq = []
                for w in range(3):
                    raw2 = pool.tile([128, 1024], F32, tag="raw2", bufs=3,
                                     name=f"raw2_{w}")
                    nc.vector.tensor_scalar(raw2[:], widesA[w][:], CLIP, -CLIP,
                                            mn, mx)
                    rawsq += [raw2[:, 0:512], raw2[:, 512:1024]]

                def rope(j, raw, eng=None):
                    eng = eng or nc.gpsimd
                    xr = pool.tile([128, 512], F32, tag="xr", bufs=3)
                    # SBUF->SBUF partition swap issued from the POOL queue:
                    # keeps the sync queue free so pass B's wqb prefetch
                    # isn't blocked behind 12 swap issues
                    nc.gpsimd.dma_start(xr[0:64, :], raw[64:128, :])
                    nc.gpsimd.dma_start(xr[64:128, :], raw[0:64, :])
                    cosT = cosq if j < HPC else cosk
                    sinT = sinq if j < HPC else sink
                    dst = qkT[:, j, tsl]
                    eng.tensor_tensor(dst, raw, cosT[:, tsl], mult)
                    eng.tensor_tensor(xr[:], xr[:], sinT[:, tsl], mult)
                    eng.tensor_tensor(dst, dst, xr[:], add)

                for j in range(6):
                    rope(j, rawsq[j])
                # pass B: k and v from the slab; previous chunk's chains
                # interleave here
                k_ps = psum.tile([128, 512], F32, tag="bank", bufs=2)
                v_ps = psum.tile([128, 512], F32, tag="bank", bufs=2)
                due = 0.0
                n_y = HPC * ((4 * (tcx - 1) + 4) // 2 + 1) if tcx >= 1 else 0
                rate = n_y / DCH if interleave is not None else 0.0
                for g in range(DG):
                    g4 = slice(g * 512, (g + 1) * 512)
                    wqb = pool.tile([128, 4, 256], F16, tag="wqb", bufs=2)
                    nc.sync.dma_start(
                        wqb[:], wqkvT_d[g4, 768:1024].rearrange(
                            "(a p) w -> p a w", p=128))
                    for i in range(4):
                        d = g * 4 + i
                        st, sp = d == 0, d == DCH - 1
                        nc.tensor.matmul(k_ps[:], wqb[:, i, 0:128],
                                         hslab[:, d, :], start=st, stop=sp)
                        for s in range(4):
                            # packed quarter-bank outputs: start=True zeroes
                            # the whole 2KB zero-region, so only the first
                            # sub-matmul of the bank may set it
                            nc.tensor.matmul(v_ps[:, s * 128:(s + 1) * 128],
                                             hslab[:, d, s * 128:(s + 1) * 128],
                                             wqb[:, i, 128:256],
                                             start=(st and s == 0),
                                             stop=(sp and s == 3),
                                             skip_group_check=True)
                        due += rate
                        while due >= 1.0:
                            next(interleave, None)
                            due -= 1.0
                if interleave is not None:
                    for _ in interleave:
                        pass
                # evac B
                rawk = pool.tile([128, 512], F32, tag="raw", bufs=2)
                nc.vector.tensor_scalar(rawk[:], k_ps[:], CLIP, -CLIP, mn, mx)
                nc.vector.tensor_scalar(
                    v_sb[:, tcx * 4:(tcx + 1) * 4, :],
                    v_ps[:].rearrange("p (a h) -> p a h", a=4),
                    CLIP, -CLIP, mn, mx)
                rope(HPC, rawk[:],
                     eng=nc.vector if tcx == TCH - 1 else None)

            def attn_chain(h, jc):
                # generator: yields once per 2-kt block so the driver can
                # interleave ready out-proj matmuls into the in-order PE
                # stream (fills the PE bubble left by the ACT-paced exp).
                # Scores for a kt-pair land in one 2-bank "wide" PSUM tile so
                # a single 1024-wide exp serves both (less ACT overhead).
                # Diagonal-straddle kt tiles (r = kt-4jc >= 0) compute only
                # the causally-needed q-suffix [128r:512] — 15% less
                # score/v PE work; the in-tile triangle is masked by one
                # [128,128] pattern at the suffix head. The unwritten prefix
                # of those PSUM halves holds stale data; exp covers it but
                # nothing downstream reads it.
                qsl = slice(jc * 512, (jc + 1) * 512)
                n_kt = 4 * jc + 4
                n_b = n_kt // 2
                attnw = psum.tile([128, 1024], F32, tag="attnw", bufs=1)
                attn_ps = attnw[:, 0:512]
                LEAD = 1
                pbs = {}
                # row sums accumulate on DVE in bf16 (2-byte dtype gets the
                # fast DVE mode); suffix-kt adds land in leg 0 (always fully
                # initialized by kt 0), full-width kts alternate legs
                two_legs = jc >= 1
                accs = [pool.tile([128, 512], BF16, tag=f"acc{i}", bufs=1,
                                  name=f"acc{i}")
                        for i in range(2 if two_legs else 1)]
                accs = accs + accs[:1] if not two_legs else accs
                for bstep in range(n_b + LEAD):
                    if bstep < n_b:
                        b = bstep
                        sc2 = psum.tile([128, 1024], F32, tag="wide", bufs=2)
                        for half in range(2):
                            kt = 2 * b + half
                            r = kt - 4 * jc
                            off = 128 * r if r > 0 else 0
                            nc.tensor.matmul(
                                sc2[:, half * 512 + off:(half + 1) * 512],
                                qkT[:, HPC, kt * 128:(kt + 1) * 128],
                                qkT[:, h, jc * 512 + off:(jc + 1) * 512],
                                start=True, stop=True,
                                skip_group_check=True)
                        pb2 = pool.tile([128, 1024], BF16, tag="pb", bufs=3)
                        nc.scalar.activation(pb2[:], sc2[:], EXP)
                        for half in range(2):
                            kt = 2 * b + half
                            r = kt - 4 * jc
                            if r >= 0:
                                msl = slice(half * 512 + 128 * r,
                                            half * 512 + 128 * r + 128)
                                nc.vector.tensor_tensor(
                                    pb2[:, msl], pb2[:, msl], masks[:], mult)
                        for half in range(2):
                            kt = 2 * b + half
                            r = kt - 4 * jc
                            if r > 0:
                                with nc.allow_low_precision(
                                        reason="bf16 row-sum legs"):
                                    nc.vector.tensor_tensor(
                                        accs[0][:, 128 * r:512],
                                        accs[0][:, 128 * r:512],
                                        pb2[:, half * 512 + 128 * r:
                                            (half + 1) * 512], add)
                            else:
                                leg = accs[kt % 2] if two_legs else accs[0]
                                psl = pb2[:, half * 512:(half + 1) * 512]
                                if kt < 2:
                                    nc.vector.tensor_scalar(
                                        leg[:], psl, 0.0, None, add)
                                else:
                                    with nc.allow_low_precision(
                                            reason="bf16 row-sum legs"):
                                        nc.vector.tensor_tensor(
                                            leg[:], leg[:], psl, add)
                        pbs[b] = pb2
                    if bstep >= LEAD:
                        b = bstep - LEAD
                        pb2 = pbs.pop(b)
                        for half in range(2):
                            kt = 2 * b + half
                            r = kt - 4 * jc
                            off = 128 * r if r > 0 else 0
                            st, sp = kt == 0, kt == n_kt - 1
                            nc.tensor.matmul(
                                attnw[:, off:512], v_sb[:, kt, :],
                                pb2[:, half * 512 + off:(half + 1) * 512],
                                start=st, stop=sp, skip_group_check=True)
                    yield
                if two_legs:
                    with nc.allow_low_precision(
                            reason="bf16 row-sum combine, 2e-2 budget"):
                        nc.vector.tensor_tensor(accs[0][:], accs[0][:],
                                                accs[1][:], add)
                # row sums via POOL partition all-reduce (fp32 internal,
                # broadcast to all partitions for free) — no PE rows spent;
                # normalize off the critical path, all-bf16 for fast DVE
                au = pool.tile([128, 512], BF16, tag="au", bufs=4)
                nc.scalar.copy(au[:], attnw[:, 0:512])
                allsum = pool.tile([128, 512], BF16, tag="rec", bufs=4)
                nc.gpsimd.partition_all_reduce(allsum[:], accs[0][:], 128,
                                               bass_isa.ReduceOp.add)
                recb = pool.tile([128, 512], BF16, tag="recb", bufs=4)
                with nc.allow_low_precision(
                        reason="bf16 softmax scale, 2e-2 budget"):
                    nc.vector.reciprocal(recb[:], allsum[:])
                nc.vector.tensor_tensor(attnT[:, h, qsl], au[:], recb[:], mult)

            def outproj_blocks(jcs):
                # flat generator of out-proj (oc, tt) blocks across t-groups
                # `jcs`; drained one block per chain step so PE never idles
                # while exp paces the chains. The wo weight tile for the
                # first (jc, oc) is DMA'd eagerly at generator creation and
                # each following one is prefetched a step ahead, so no block
                # ever waits on its weight transfer. PSUM->SBUF evacs
                # alternate ACT/DVE (GPSIMD cannot read PSUM).
                # oc-major across the t-groups: one wo load serves every
                # group's blocks for that column chunk (3x less weight DMA
                # on the interleaved portion)
                pairs = [(jc, oc) for oc in range(OCH) for jc in jcs]

                def load_wo(oc):
                    wo = pool.tile([128, ICH, 512], F16, tag="wo", bufs=3)
                    osl = slice(oc * 512, (oc + 1) * 512)
                    nc.sync.dma_start(wo[:], woutT_d[:, osl].rearrange(
                        "(i p) o -> p i o", p=128))
                    return wo

                pending = [load_wo(pairs[0][1])]

                def gen():
                    wo = None
                    last_oc = None
                    for n, (jc, oc) in enumerate(pairs):
                        if oc != last_oc:
                            nxt = next((p[1] for p in pairs[n + 1:]
                                        if p[1] != oc), None)
                            if nxt is not None:
                                pending.append(load_wo(nxt))
                            wo = pending.pop(0)
                            last_oc = oc
                        osl = slice(oc * 512, (oc + 1) * 512)
                        for tt in range(4):
                            t = 4 * jc + tt
                            out_ps = psum.tile([128, 512], F32, tag="bank",
                                               bufs=2)
                            for i in range(ICH):
                                nc.tensor.matmul(
                                    out_ps[:],
                                    attnT[:, i, t * 128:(t + 1) * 128],
                                    wo[:, i, :], start=(i == 0),
                                    stop=(i == ICH - 1))
                            osb = pool.tile([128, 512], F16, tag="osb", bufs=4)
                            th = slice(jc * 512 + tt * 128,
                                       jc * 512 + (tt + 1) * 128)
                            if (oc + tt) % 2 == 0:
                                nc.scalar.copy(osb[:], out_ps[:])
                            else:
                                nc.vector.tensor_copy(osb[:], out_ps[:])
                            nc.sync.dma_start(outp_d[th, osl], osb[:])
                            yield

                return gen()

            # ---- Sweeps with the previous chunk's chains interleaved
            # into pass B; post-QKV: last chunk's chains with all out-proj
            # groups interleaved into the PE stream ----
            def chain_group(jc):
                for h in range(HPC):
                    for _ in attn_chain(h, jc):
                        yield

            load_tables()
            qkv_sweep(0)
            for tcx in range(1, TCH):
                qkv_sweep(tcx, interleave=chain_group(tcx - 1))
            ops = outproj_blocks(list(range(TCH - 1)))
            due = 0.0
            rate = (3 * 4 * OCH) / (HPC * ((4 * 3 + 4) // 2 + 1))
            for h in range(HPC):
                for _ in attn_chain(h, TCH - 1):
                    due += rate
                    while due >= 1.0:
                        next(ops, None)
                        due -= 1.0
            # create the last group's generator before draining the rest so
            # its first weight tile is already in flight
            tail = outproj_blocks([TCH - 1])
            for _ in ops:
                pass
            for _ in tail:
                pass

    nc.compile()
    return nc


def kernel(hidden_states, position_ids, Wqkv, Wout):
    global _compiled
    hidden_states = np.asarray(hidden_states, dtype=np.float32)
    position_ids = np.asarray(position_ids).astype(np.int64)
    Wqkv = np.asarray(Wqkv, dtype=np.float32)
    Wout = np.asarray(Wout, dtype=np.float32)

    if _compiled is None:
        _compiled = _build()
    nc = _compiled

    # host prep: rope tables (from actual position_ids), masks, shards
    scale = HD ** -0.5
    half = HD // 2
    inv_freq = 1.0 / (THETA ** (np.arange(half, dtype=np.float64) / half))
    freqs = position_ids.astype(np.float64)[None, :] * inv_freq[:, None]  # [64, T]
    cos = np.cos(freqs)
    sin = np.sin(freqs)
    cosf = np.concatenate([cos, cos], 0)
    sinf = np.concatenate([-sin, sin], 0)
    cosq = (cosf * scale).astype(np.float16)
    sinq = (sinf * scale).astype(np.float16)
    cosk = cosf.astype(np.float16)
    sink = sinf.astype(np.float16)

    p = np.arange(128)[:, None]
    f = np.arange(128)[None, :]
    masks = (f >= p).astype(np.float16)

    hidT = np.ascontiguousarray(hidden_states.T).astype(np.float16)

    q_size = N_HEADS * HD
    in_maps = []
    for c in range(N_CORES):
        qrows = Wqkv[c * HPC * HD:(c + 1) * HPC * HD]
        krows = Wqkv[q_size + c * HD:q_size + (c + 1) * HD]
        vrows = Wqkv[q_size + N_KV * HD + c * HD:q_size + N_KV * HD + (c + 1) * HD]
        wqkvT = np.ascontiguousarray(
            np.concatenate([qrows, krows, vrows], 0).T).astype(np.float16)
        woutT = np.ascontiguousarray(
            Wout[:, c * HPC * HD:(c + 1) * HPC * HD].T).astype(np.float16)
        in_maps.append({
            "hidT": hidT, "wqkvT": wqkvT, "woutT": woutT,
            "cosq": cosq, "sinq": sinq, "cosk": cosk, "sink": sink,
            "maskm": masks,
        })

    trace = os.environ.get("DBRX_TRACE", "0") == "1"
    res = run_bass_kernel_spmd(nc, in_maps, core_ids=list(range(N_CORES)),
                               trace=trace)
    kernel.last_result = res

    out = res.results[0]["outp"].astype(np.float32)
    for c in range(1, N_CORES):
        out += res.results[c]["outp"].astype(np.float32)
    return out



# revision 9
# speedup vs baseline: 1.1103x; 1.1103x over previous
r"""DbrxAttention on 8 TRN2 NeuronCores, tensor-parallel across heads.

Per-core shard (core c of 8): 6 query heads (q heads 6c..6c+5), kv head c
(replicated per its 6-head query group), plus the matching 768 input
columns of the out-projection. Each core computes a partial out-proj
(row-parallel Wout); the partials are summed on the host (the all-reduce
of the TP pattern).

Layouts (per core, all device tensors):
  hidT   [6144, 2048] fp16  hidden^T       (d on partitions)
  wqkvT  [6144, 1024] fp16  [q0..q5 | k | v] columns of Wqkv^T shard
  woutT  [768,  6144] fp16  Wout[:, shard]^T
  cos/sin tables [128, 2048] fp16, neox rope with sign-folded sin and the
  1/sqrt(128) score scale folded into the q tables.
  masks  [128, 128] fp16  multiplicative causal mask (f >= p) for the
         in-tile triangle of diagonal score tiles

Structure (PE runs ~99% busy in the schedule sim):
- Each QKV sweep (512-t chunk) is two passes over the 48 d-chunks reading
  a persistent SBUF hid slab: pass A computes q0..q5 in three 2-bank
  "wide" PSUM tiles; pass B computes k + v in the 2-bank ring. During
  pass B the wides are free, so the previous chunk's attention chains
  interleave into the in-order PE stream (their exp/DVE latency hides
  under k/v GEMM work).
- Chains (per head, 512-q chunk): kt-pair score matmuls into one wide
  PSUM tile -> single 1024-wide exp on ACT -> bf16 probs; diagonal kt
  tiles compute only the causal q-suffix (15% less score/v work) with a
  single 128x128 triangle mask on DVE; softmax row sums accumulate on
  DVE in bf16 legs + one POOL partition_all_reduce per chain (fp32
  internal, result broadcast to all partitions for free); attention
  accumulates in the "attnw" wide PSUM tile; normalization is all-bf16
  (ACT copy, DVE reciprocal, DVE multiply).
- The last chunk's chains run after QKV with every out-proj group's
  (oc, tt) blocks rate-interleaved into the PE stream; out-proj PSUM
  evacuations alternate ACT/DVE (GPSIMD cannot read PSUM) and partial
  fp16 outputs stream per t-tile. Host sums the 8 partials in fp32.
- Queue routing keeps the in-order DMA queues unblocked: hid slab on
  ACT, weights on SP, rope partition-swaps + tables on POOL; the last
  sweep's k-rope runs on DVE so the final chains aren't queued behind
  the q-ropes on POOL.
"""

import os

import numpy as np

import concourse.mybir as mybir
import concourse.tile as tile
from concourse import bacc
from concourse import bass_isa
from concourse.bass_utils import run_bass_kernel_spmd

F32R = mybir.dt.float32r
F32 = mybir.dt.float32
F16 = mybir.dt.float16
BF16 = mybir.dt.bfloat16
F8 = mybir.dt.float8e4
DRM = mybir.MatmulPerfMode.DoubleRow

T = 2048
D = 6144
N_HEADS = 48
N_KV = 8
HD = 128
CLIP = 8.0
THETA = 500000.0
N_CORES = 8
HPC = N_HEADS // N_CORES      # q heads per core = 6
QKJ = HPC + 1                 # q+k j-tiles per core = 7
DCH = D // 128                # 48 contraction chunks
DG = DCH // 4                 # 12 batched (4-chunk) DMA groups
TCH = T // 512                # 4 t-chunks
TTILES = T // 128             # 16 t-tiles
OCH = D // 512                # 12 out-proj column chunks
ICH = HPC                     # 6 out-proj contraction chunks (768/128)
A_SCALE = 32.0                # host pre-scale on Wqkv so fp8(e4m3) hi/lo
                              # splits of the 0.02-sigma weights stay out of
                              # the denormal floor; compensated in the rope
                              # tables (q: score_scale/A, k: 1/A), the clip
                              # constants (8*A), and a final host divide for
                              # the v path.

_compiled = None


def _build():
    nc = bacc.Bacc("TRN2", target_bir_lowering=False, debug=False,
                   num_devices=N_CORES)

    hidh_d = nc.dram_tensor("hidh", [D, T], F8, kind="ExternalInput").ap()
    hidl_d = nc.dram_tensor("hidl", [D, T], F8, kind="ExternalInput").ap()
    wqh_d = nc.dram_tensor("wqh", [D, 1024], F8, kind="ExternalInput").ap()
    wql_d = nc.dram_tensor("wql", [D, 1024], F8, kind="ExternalInput").ap()
    woutT_d = nc.dram_tensor("woutT", [HPC * HD, D], F16, kind="ExternalInput").ap()
    cosq_d = nc.dram_tensor("cosq", [HD, T], F16, kind="ExternalInput").ap()
    sinq_d = nc.dram_tensor("sinq", [HD, T], F16, kind="ExternalInput").ap()
    cosk_d = nc.dram_tensor("cosk", [HD, T], F16, kind="ExternalInput").ap()
    sink_d = nc.dram_tensor("sink", [HD, T], F16, kind="ExternalInput").ap()
    mask_d = nc.dram_tensor("maskm", [HD, 128], F16, kind="ExternalInput").ap()
    outp_d = nc.dram_tensor("outp", [T, D], F16, kind="ExternalOutput").ap()

    mn, mx = mybir.AluOpType.min, mybir.AluOpType.max
    mult, add = mybir.AluOpType.mult, mybir.AluOpType.add
    EXP = mybir.ActivationFunctionType.Exp

    with tile.TileContext(nc) as tc:
        with (
            tc.tile_pool(name="sb", bufs=1) as pool,
            tc.tile_pool(name="ps", bufs=1, space="PSUM") as psum,
        ):
            # persistent tensors
            qkT = pool.tile([128, QKJ, T], F16)       # roped q (scaled) + k
            v_sb = pool.tile([128, TTILES, HD], BF16)  # clipped v, [t%128, t//128, hd]
            attnT = pool.tile([128, HPC, T], F16)      # normalized attn^T
            cosq = pool.tile([HD, T], F16)
            sinq = pool.tile([HD, T], F16)
            cosk = pool.tile([HD, T], F16)
            sink = pool.tile([HD, T], F16)
            masks = pool.tile([HD, 128], F16)
            # persistent hid slab: per-sweep writes overwrite slices, so the
            # WAR against the previous sweep's readers is tracked per-slice
            # (a per-sweep pool.tile would bump the whole-tile version and
            # serialize the refill behind all of pass B).
            # fp8 hi planes 0..DCH-1, lo planes DCH..2*DCH-1 — every
            # DoubleRow pair is a contiguous [:, a:a+2, :] slice.
            hslab = pool.tile([128, 2 * DCH, 512], F8)

            def load_tables():
                nc.gpsimd.dma_start(cosq[:], cosq_d[:])
                nc.gpsimd.dma_start(sinq[:], sinq_d[:])
                nc.gpsimd.dma_start(cosk[:], cosk_d[:])
                nc.gpsimd.dma_start(sink[:], sink_d[:])
                nc.gpsimd.dma_start(masks[:], mask_d[:])

            def qkv_sweep(tcx, interleave=None):
                # Pass A: q heads j0..j5 accumulate in three 2-bank "wide"
                # PSUM tiles (2 from the sc2 ring + the attnw tile) over all
                # 48 d-chunks; hid lands in a resident SBUF slab. Pass B:
                # k (j6) and v accumulate in the 2-bank ring re-reading the
                # slab (no second hid DMA). During pass B the wides are free
                # again, so the previous chunk's attention chains interleave
                # into the PE stream (hiding the chain latency that
                # otherwise pays off only after the last sweep).
                # All projection matmuls run as fp8(e4m3) DoubleRow 3-term
                # compensation (hi*hi + hi*lo + lo*hi): each DR instruction
                # contracts a contiguous pair of 128-d units at 0.5
                # cycles/row, so a d-pair costs 3 DRs vs 2 fp16 matmuls =
                # 0.75x PE time at ~0.15% error.
                tsl = slice(tcx * 512, (tcx + 1) * 512)
                widesA = [psum.tile([128, 1024], F32, tag="wide", bufs=2,
                                    name=f"qkw{w}") for w in range(2)]
                widesA.append(psum.tile([128, 1024], F32, tag="attnw", bufs=1,
                                        name="qkw2"))
                qk_ps = [widesA[j // 2][:, (j % 2) * 512:(j % 2 + 1) * 512]
                         for j in range(6)]
                for g in range(DG):
                    g4 = slice(g * 512, (g + 1) * 512)
                    wqa = pool.tile([128, 8, 768], F8, tag="wqa", bufs=3)
                    if tcx == 0 and g == 0:
                        for i in range(4):
                            dsl = slice(i * 128, (i + 1) * 128)
                            nc.sync.dma_start(wqa[:, i, :], wqh_d[dsl, 0:768])
                            nc.scalar.dma_start(hslab[:, i, :],
                                                hidh_d[dsl, tsl])
                        for i in range(4):
                            dsl = slice(i * 128, (i + 1) * 128)
                            nc.sync.dma_start(wqa[:, 4 + i, :],
                                              wql_d[dsl, 0:768])
                            nc.scalar.dma_start(hslab[:, DCH + i, :],
                                                hidl_d[dsl, tsl])
                    else:
                        nc.scalar.dma_start(
                            hslab[:, g * 4:(g + 1) * 4, :],
                            hidh_d[g4, tsl].rearrange("(a p) t -> p a t",
                                                      p=128))
                        nc.scalar.dma_start(
                            hslab[:, DCH + g * 4:DCH + (g + 1) * 4, :],
                            hidl_d[g4, tsl].rearrange("(a p) t -> p a t",
                                                      p=128))
                        nc.sync.dma_start(
                            wqa[:, 0:4, :], wqh_d[g4, 0:768].rearrange(
                                "(a p) w -> p a w", p=128))
                        nc.sync.dma_start(
                            wqa[:, 4:8, :], wql_d[g4, 0:768].rearrange(
                                "(a p) w -> p a w", p=128))
                    for i2 in range(2):
                        d = g * 4 + 2 * i2
                        wh = slice(2 * i2, 2 * i2 + 2)
                        wl = slice(4 + 2 * i2, 4 + 2 * i2 + 2)
                        hh = slice(d, d + 2)
                        hl = slice(DCH + d, DCH + d + 2)
                        st, sp = d == 0, d == DCH - 2
                        for j in range(6):
                            jsl = slice(j * 128, (j + 1) * 128)
                            nc.tensor.matmul(qk_ps[j], wqa[:, wh, jsl],
                                             hslab[:, hh, :], start=st,
                                             stop=False, perf_mode=DRM,
                                             skip_group_check=True)
                            nc.tensor.matmul(qk_ps[j], wqa[:, wh, jsl],
                                             hslab[:, hl, :], start=False,
                                             stop=False, perf_mode=DRM,
                                             skip_group_check=True)
                            nc.tensor.matmul(qk_ps[j], wqa[:, wl, jsl],
                                             hslab[:, hh, :], start=False,
                                             stop=sp, perf_mode=DRM,
                                             skip_group_check=True)
                # evac A: clips first (release the wides for the interleaved
                # chains), then ropes for q0..q5 on POOL
                rawsq = []
                for w in range(3):
                    raw2 = pool.tile([128, 1024], F32, tag="raw2", bufs=3,
                                     name=f"raw2_{w}")
                    nc.vector.tensor_scalar(raw2[:], widesA[w][:],
                                            CLIP * A_SCALE, -CLIP * A_SCALE,
                                            mn, mx)
                    rawsq += [raw2[:, 0:512], raw2[:, 512:1024]]

                def rope(j, raw, eng=None):
                    eng = eng or nc.gpsimd
                    xr = pool.tile([128, 512], F32, tag="xr", bufs=3)
                    # SBUF->SBUF partition swap issued from the POOL queue:
                    # keeps the sync queue free so pass B's wqb prefetch
                    # isn't blocked behind 12 swap issues
                    nc.gpsimd.dma_start(xr[0:64, :], raw[64:128, :])
                    nc.gpsimd.dma_start(xr[64:128, :], raw[0:64, :])
                    cosT = cosq if j < HPC else cosk
                    sinT = sinq if j < HPC else sink
                    dst = qkT[:, j, tsl]
                    eng.tensor_tensor(dst, raw, cosT[:, tsl], mult)
                    eng.tensor_tensor(xr[:], xr[:], sinT[:, tsl], mult)
                    eng.tensor_tensor(dst, dst, xr[:], add)

                for j in range(6):
                    rope(j, rawsq[j])
                # pass B: k and v from the slab; previous chunk's chains
                # interleave here
                k_ps = psum.tile([128, 512], F32, tag="bank", bufs=2)
                v_ps = psum.tile([128, 512], F32, tag="bank", bufs=2)
                due = 0.0
                n_y = HPC * ((4 * (tcx - 1) + 4) // 2 + 1) if tcx >= 1 else 0
                rate = n_y / (2 * DG) if interleave is not None else 0.0
                for g in range(DG):
                    g4 = slice(g * 512, (g + 1) * 512)
                    wqb = pool.tile([128, 8, 256], F8, tag="wqb", bufs=2)
                    nc.sync.dma_start(
                        wqb[:, 0:4, :], wqh_d[g4, 768:1024].rearrange(
                            "(a p) w -> p a w", p=128))
                    nc.sync.dma_start(
                        wqb[:, 4:8, :], wql_d[g4, 768:1024].rearrange(
                            "(a p) w -> p a w", p=128))
                    for i2 in range(2):
                        d = g * 4 + 2 * i2
                        wh = slice(2 * i2, 2 * i2 + 2)
                        wl = slice(4 + 2 * i2, 4 + 2 * i2 + 2)
                        hh = slice(d, d + 2)
                        hl = slice(DCH + d, DCH + d + 2)
                        st, sp = d == 0, d == DCH - 2
                        nc.tensor.matmul(k_ps[:], wqb[:, wh, 0:128],
                                         hslab[:, hh, :], start=st,
                                         stop=False, perf_mode=DRM)
                        nc.tensor.matmul(k_ps[:], wqb[:, wh, 0:128],
                                         hslab[:, hl, :], start=False,
                                         stop=False, perf_mode=DRM)
                        nc.tensor.matmul(k_ps[:], wqb[:, wl, 0:128],
                                         hslab[:, hh, :], start=False,
                                         stop=sp, perf_mode=DRM)
                        for s in range(4):
                            # packed quarter-bank outputs: start=True zeroes
                            # the whole 2KB zero-region, so only the first
                            # sub-matmul of the bank may set it
                            s128 = slice(s * 128, (s + 1) * 128)
                            nc.tensor.matmul(v_ps[:, s128],
                                             hslab[:, hh, s128],
                                             wqb[:, wh, 128:256],
                                             start=(st and s == 0),
                                             stop=False, perf_mode=DRM,
                                             skip_group_check=True)
                            nc.tensor.matmul(v_ps[:, s128],
                                             hslab[:, hh, s128],
                                             wqb[:, wl, 128:256],
                                             start=False, stop=False,
                                             perf_mode=DRM,
                                             skip_group_check=True)
                            nc.tensor.matmul(v_ps[:, s128],
                                             hslab[:, hl, s128],
                                             wqb[:, wh, 128:256],
                                             start=False,
                                             stop=(sp and s == 3),
                                             perf_mode=DRM,
                                             skip_group_check=True)
                        due += rate
                        while due >= 1.0:
                            next(interleave, None)
                            due -= 1.0
                if interleave is not None:
                    for _ in interleave:
                        pass
                # evac B
                rawk = pool.tile([128, 512], F32, tag="raw", bufs=2)
                nc.vector.tensor_scalar(rawk[:], k_ps[:], CLIP * A_SCALE,
                                        -CLIP * A_SCALE, mn, mx)
                nc.vector.tensor_scalar(
                    v_sb[:, tcx * 4:(tcx + 1) * 4, :],
                    v_ps[:].rearrange("p (a h) -> p a h", a=4),
                    CLIP * A_SCALE, -CLIP * A_SCALE, mn, mx)
                rope(HPC, rawk[:],
                     eng=nc.vector if tcx == TCH - 1 else None)

            def attn_chain(h, jc):
                # generator: yields once per 2-kt block so the driver can
                # interleave ready out-proj matmuls into the in-order PE
                # stream (fills the PE bubble left by the ACT-paced exp).
                # Scores for a kt-pair land in one 2-bank "wide" PSUM tile so
                # a single 1024-wide exp serves both (less ACT overhead).
                # Diagonal-straddle kt tiles (r = kt-4jc >= 0) compute only
                # the causally-needed q-suffix [128r:512] — 15% less
                # score/v PE work; the in-tile triangle is masked by one
                # [128,128] pattern at the suffix head. The unwritten prefix
                # of those PSUM halves holds stale data; exp covers it but
                # nothing downstream reads it.
                qsl = slice(jc * 512, (jc + 1) * 512)
                n_kt = 4 * jc + 4
                n_b = n_kt // 2
                attnw = psum.tile([128, 1024], F32, tag="attnw", bufs=1)
                attn_ps = attnw[:, 0:512]
                LEAD = 1
                pbs = {}
                # row sums accumulate on DVE in bf16 (2-byte dtype gets the
                # fast DVE mode); suffix-kt adds land in leg 0 (always fully
                # initialized by kt 0), full-width kts alternate legs
                two_legs = jc >= 1
                accs = [pool.tile([128, 512], BF16, tag=f"acc{i}", bufs=1,
                                  name=f"acc{i}")
                        for i in range(2 if two_legs else 1)]
                accs = accs + accs[:1] if not two_legs else accs
                for bstep in range(n_b + LEAD):
                    if bstep < n_b:
                        b = bstep
                        sc2 = psum.tile([128, 1024], F32, tag="wide", bufs=2)
                        for half in range(2):
                            kt = 2 * b + half
                            r = kt - 4 * jc
                            off = 128 * r if r > 0 else 0
                            nc.tensor.matmul(
                                sc2[:, half * 512 + off:(half + 1) * 512],
                                qkT[:, HPC, kt * 128:(kt + 1) * 128],
                                qkT[:, h, jc * 512 + off:(jc + 1) * 512],
                                start=True, stop=True,
                                skip_group_check=True)
                        pb2 = pool.tile([128, 1024], BF16, tag="pb", bufs=3)
                        nc.scalar.activation(pb2[:], sc2[:], EXP)
                        for half in range(2):
                            kt = 2 * b + half
                            r = kt - 4 * jc
                            if r >= 0:
                                msl = slice(half * 512 + 128 * r,
                                            half * 512 + 128 * r + 128)
                                nc.vector.tensor_tensor(
                                    pb2[:, msl], pb2[:, msl], masks[:], mult)
                        for half in range(2):
                            kt = 2 * b + half
                            r = kt - 4 * jc
                            if r > 0:
                                with nc.allow_low_precision(
                                        reason="bf16 row-sum legs"):
                                    nc.vector.tensor_tensor(
                                        accs[0][:, 128 * r:512],
                                        accs[0][:, 128 * r:512],
                                        pb2[:, half * 512 + 128 * r:
                                            (half + 1) * 512], add)
                            else:
                                leg = accs[kt % 2] if two_legs else accs[0]
                                psl = pb2[:, half * 512:(half + 1) * 512]
                                if kt < 2:
                                    nc.vector.tensor_scalar(
                                        leg[:], psl, 0.0, None, add)
                                else:
                                    with nc.allow_low_precision(
                                            reason="bf16 row-sum legs"):
                                        nc.vector.tensor_tensor(
                                            leg[:], leg[:], psl, add)
                        pbs[b] = pb2
                    if bstep >= LEAD:
                        b = bstep - LEAD
                        pb2 = pbs.pop(b)
                        for half in range(2):
                            kt = 2 * b + half
                            r = kt - 4 * jc
                            off = 128 * r if r > 0 else 0
                            st, sp = kt == 0, kt == n_kt - 1
                            nc.tensor.matmul(
                                attnw[:, off:512], v_sb[:, kt, :],
                                pb2[:, half * 512 + off:(half + 1) * 512],
                                start=st, stop=sp, skip_group_check=True)
                    yield
                if two_legs:
                    with nc.allow_low_precision(
                            reason="bf16 row-sum combine, 2e-2 budget"):
                        nc.vector.tensor_tensor(accs[0][:], accs[0][:],
                                                accs[1][:], add)
                # row sums via POOL partition all-reduce (fp32 internal,
                # broadcast to all partitions for free) — no PE rows spent;
                # normalize off the critical path, all-bf16 for fast DVE
                au = pool.tile([128, 512], BF16, tag="au", bufs=4)
                nc.scalar.copy(au[:], attnw[:, 0:512])
                allsum = pool.tile([128, 512], BF16, tag="rec", bufs=4)
                nc.gpsimd.partition_all_reduce(allsum[:], accs[0][:], 128,
                                               bass_isa.ReduceOp.add)
                recb = pool.tile([128, 512], BF16, tag="recb", bufs=4)
                with nc.allow_low_precision(
                        reason="bf16 softmax scale, 2e-2 budget"):
                    nc.vector.reciprocal(recb[:], allsum[:])
                nc.vector.tensor_tensor(attnT[:, h, qsl], au[:], recb[:], mult)

            def outproj_blocks(jcs):
                # flat generator of out-proj (oc, tt) blocks across t-groups
                # `jcs`; drained one block per chain step so PE never idles
                # while exp paces the chains. The wo weight tile for the
                # first (jc, oc) is DMA'd eagerly at generator creation and
                # each following one is prefetched a step ahead, so no block
                # ever waits on its weight transfer. PSUM->SBUF evacs
                # alternate ACT/DVE (GPSIMD cannot read PSUM).
                # oc-major across the t-groups: one wo load serves every
                # group's blocks for that column chunk (3x less weight DMA
                # on the interleaved portion)
                pairs = [(jc, oc) for oc in range(OCH) for jc in jcs]

                def load_wo(oc):
                    wo = pool.tile([128, ICH, 512], F16, tag="wo", bufs=3)
                    osl = slice(oc * 512, (oc + 1) * 512)
                    nc.sync.dma_start(wo[:], woutT_d[:, osl].rearrange(
                        "(i p) o -> p i o", p=128))
                    return wo

                pending = [load_wo(pairs[0][1])]

                def gen():
                    wo = None
                    last_oc = None
                    for n, (jc, oc) in enumerate(pairs):
                        if oc != last_oc:
                            nxt = next((p[1] for p in pairs[n + 1:]
                                        if p[1] != oc), None)
                            if nxt is not None:
                                pending.append(load_wo(nxt))
                            wo = pending.pop(0)
                            last_oc = oc
                        osl = slice(oc * 512, (oc + 1) * 512)
                        for tt in range(4):
                            t = 4 * jc + tt
                            out_ps = psum.tile([128, 512], F32, tag="bank",
                                               bufs=2)
                            for i in range(ICH):
                                nc.tensor.matmul(
                                    out_ps[:],
                                    attnT[:, i, t * 128:(t + 1) * 128],
                                    wo[:, i, :], start=(i == 0),
                                    stop=(i == ICH - 1))
                            osb = pool.tile([128, 512], F16, tag="osb", bufs=4)
                            th = slice(jc * 512 + tt * 128,
                                       jc * 512 + (tt + 1) * 128)
                            if (oc + tt) % 2 == 0:
                                nc.scalar.copy(osb[:], out_ps[:])
                            else:
                                nc.vector.tensor_copy(osb[:], out_ps[:])
                            nc.sync.dma_start(outp_d[th, osl], osb[:])
                            yield

                return gen()

            # ---- Sweeps with the previous chunk's chains interleaved
            # into pass B; post-QKV: last chunk's chains with all out-proj
            # groups interleaved into the PE stream ----
            def chain_group(jc):
                for h in range(HPC):
                    for _ in attn_chain(h, jc):
                        yield

            load_tables()
            qkv_sweep(0)
            for tcx in range(1, TCH):
                qkv_sweep(tcx, interleave=chain_group(tcx - 1))
            ops = outproj_blocks(list(range(TCH - 1)))
            due = 0.0
            rate = (3 * 4 * OCH) / (HPC * ((4 * 3 + 4) // 2 + 1))
            for h in range(HPC):
                for _ in attn_chain(h, TCH - 1):
                    due += rate
                    while due >= 1.0:
                        next(ops, None)
                        due -= 1.0
            # create the last group's generator before draining the rest so
            # its first weight tile is already in flight
            tail = outproj_blocks([TCH - 1])
            for _ in ops:
                pass
            for _ in tail:
                pass

    nc.compile()
    return nc


def kernel(hidden_states, position_ids, Wqkv, Wout):
    global _compiled
    hidden_states = np.asarray(hidden_states, dtype=np.float32)
    position_ids = np.asarray(position_ids).astype(np.int64)
    Wqkv = np.asarray(Wqkv, dtype=np.float32)
    Wout = np.asarray(Wout, dtype=np.float32)

    if _compiled is None:
        _compiled = _build()
    nc = _compiled

    import ml_dtypes
    E4M3 = ml_dtypes.float8_e4m3

    def split8(x):
        hi = x.astype(E4M3)
        lo = (x - hi.astype(np.float32)).astype(E4M3)
        return np.ascontiguousarray(hi), np.ascontiguousarray(lo)

    # host prep: rope tables (from actual position_ids), masks, shards.
    # Wqkv is pre-scaled by A_SCALE for the fp8 split; the q tables fold
    # score_scale/A_SCALE, the k tables 1/A_SCALE, and the v path's factor
    # is divided out of the final host sum.
    scale = HD ** -0.5
    half = HD // 2
    inv_freq = 1.0 / (THETA ** (np.arange(half, dtype=np.float64) / half))
    freqs = position_ids.astype(np.float64)[None, :] * inv_freq[:, None]  # [64, T]
    cos = np.cos(freqs)
    sin = np.sin(freqs)
    cosf = np.concatenate([cos, cos], 0)
    sinf = np.concatenate([-sin, sin], 0)
    cosq = (cosf * (scale / A_SCALE)).astype(np.float16)
    sinq = (sinf * (scale / A_SCALE)).astype(np.float16)
    cosk = (cosf / A_SCALE).astype(np.float16)
    sink = (sinf / A_SCALE).astype(np.float16)

    p = np.arange(128)[:, None]
    f = np.arange(128)[None, :]
    masks = (f >= p).astype(np.float16)

    hidT = np.ascontiguousarray(hidden_states.T)
    hidh, hidl = split8(hidT)

    q_size = N_HEADS * HD
    in_maps = []
    for c in range(N_CORES):
        qrows = Wqkv[c * HPC * HD:(c + 1) * HPC * HD]
        krows = Wqkv[q_size + c * HD:q_size + (c + 1) * HD]
        vrows = Wqkv[q_size + N_KV * HD + c * HD:q_size + N_KV * HD + (c + 1) * HD]
        wqkvT = np.ascontiguousarray(
            np.concatenate([qrows, krows, vrows], 0).T) * A_SCALE
        wqh, wql = split8(wqkvT)
        woutT = np.ascontiguousarray(
            Wout[:, c * HPC * HD:(c + 1) * HPC * HD].T).astype(np.float16)
        in_maps.append({
            "hidh": hidh, "hidl": hidl, "wqh": wqh, "wql": wql,
            "woutT": woutT,
            "cosq": cosq, "sinq": sinq, "cosk": cosk, "sink": sink,
            "maskm": masks,
        })

    trace = os.environ.get("DBRX_TRACE", "0") == "1"
    res = run_bass_kernel_spmd(nc, in_maps, core_ids=list(range(N_CORES)),
                               trace=trace)
    kernel.last_result = res

    out = res.results[0]["outp"].astype(np.float32)
    for c in range(1, N_CORES):
        out += res.results[c]["outp"].astype(np.float32)
    # undo the v-path A_SCALE carried through attnT into the out-projection
    out /= A_SCALE
    return out



# revision 19
# speedup vs baseline: 1.2081x; 1.0881x over previous
r"""DbrxAttention on 8 TRN2 NeuronCores, tensor-parallel across heads.

Per-core shard (core c of 8): 6 query heads (q heads 6c..6c+5), kv head c
(replicated per its 6-head query group), plus the matching 768 input
columns of the out-projection. Each core computes a partial out-proj
(row-parallel Wout); the partials are summed on the host (the all-reduce
of the TP pattern).

Layouts (per core, all device tensors):
  hidT   [6144, 2048] fp16  hidden^T       (d on partitions)
  wqkvT  [6144, 1024] fp16  [q0..q5 | k | v] columns of Wqkv^T shard
  woutT  [768,  6144] fp16  Wout[:, shard]^T
  cos/sin tables [128, 2048] fp16, neox rope with sign-folded sin and the
  1/sqrt(128) score scale folded into the q tables.
  masks  [128, 128] fp16  multiplicative causal mask (f >= p) for the
         in-tile triangle of diagonal score tiles

Structure (PE runs ~99% busy in the schedule sim):
- Each QKV sweep (512-t chunk) is two passes over the 48 d-chunks reading
  a persistent SBUF hid slab: pass A computes q0..q5 in three 2-bank
  "wide" PSUM tiles; pass B computes k + v in the 2-bank ring. During
  pass B the wides are free, so the previous chunk's attention chains
  interleave into the in-order PE stream (their exp/DVE latency hides
  under k/v GEMM work).
- Chains (per head, 512-q chunk): kt-pair score matmuls into one wide
  PSUM tile -> single 1024-wide exp on ACT -> bf16 probs; diagonal kt
  tiles compute only the causal q-suffix (15% less score/v work) with a
  single 128x128 triangle mask on DVE; softmax row sums accumulate on
  DVE in bf16 legs + one POOL partition_all_reduce per chain (fp32
  internal, result broadcast to all partitions for free); attention
  accumulates in the "attnw" wide PSUM tile; normalization is all-bf16
  (ACT copy, DVE reciprocal, DVE multiply).
- The last chunk's chains run after QKV with every out-proj group's
  (oc, tt) blocks rate-interleaved into the PE stream; out-proj PSUM
  evacuations alternate ACT/DVE (GPSIMD cannot read PSUM) and partial
  fp16 outputs stream per t-tile. Host sums the 8 partials in fp32.
- Queue routing keeps the in-order DMA queues unblocked: hid slab on
  ACT, weights on SP, rope partition-swaps + tables on POOL; the last
  sweep's k-rope runs on DVE so the final chains aren't queued behind
  the q-ropes on POOL.
"""

import os

import numpy as np

import concourse.mybir as mybir
import concourse.tile as tile
from concourse import bacc
from concourse import bass_isa
from concourse.bass_utils import run_bass_kernel_spmd

F32R = mybir.dt.float32r
F32 = mybir.dt.float32
F16 = mybir.dt.float16
BF16 = mybir.dt.bfloat16
F8 = mybir.dt.float8e4
DRM = mybir.MatmulPerfMode.DoubleRow

T = 2048
D = 6144
N_HEADS = 48
N_KV = 8
HD = 128
CLIP = 8.0
THETA = 500000.0
N_CORES = 8
HPC = N_HEADS // N_CORES      # q heads per core = 6
QKJ = HPC + 1                 # q+k j-tiles per core = 7
DCH = D // 128                # 48 contraction chunks
DG = DCH // 4                 # 12 batched (4-chunk) DMA groups
TCH = T // 512                # 4 t-chunks
TTILES = T // 128             # 16 t-tiles
OCH = D // 512                # 12 out-proj column chunks
ICH = HPC                     # 6 out-proj contraction chunks (768/128)
A_SCALE = 16.0                # host pre-scale on Wqkv so fp8(e4m3) hi/lo
                              # splits of the 0.02-sigma weights stay out of
                              # the denormal floor; compensated in the rope
                              # tables (q: score_scale/A, k: 1/A), the clip
                              # constants (8*A), and a final host divide for
                              # the v path. Kept at 16 so clipped v (and thus
                              # attn) stays within e4m3's +-240 range when
                              # attnT is stored as fp8 hi/lo.
B_SCALE = 32.0                # same for Wout; the final host sum divides by
                              # A_SCALE * B_SCALE.

_compiled = None


def _build():
    nc = bacc.Bacc("TRN2", target_bir_lowering=False, debug=False,
                   num_devices=N_CORES)

    hidh_d = nc.dram_tensor("hidh", [D, T], F8, kind="ExternalInput").ap()
    hidl_d = nc.dram_tensor("hidl", [D, T], F8, kind="ExternalInput").ap()
    wqh_d = nc.dram_tensor("wqh", [D, 1024], F8, kind="ExternalInput").ap()
    wql_d = nc.dram_tensor("wql", [D, 1024], F8, kind="ExternalInput").ap()
    wouth_d = nc.dram_tensor("wouth", [HPC * HD, D], F8, kind="ExternalInput").ap()
    woutl_d = nc.dram_tensor("woutl", [HPC * HD, D], F8, kind="ExternalInput").ap()
    cosq_d = nc.dram_tensor("cosq", [HD, T], F16, kind="ExternalInput").ap()
    sinq_d = nc.dram_tensor("sinq", [HD, T], F16, kind="ExternalInput").ap()
    cosk_d = nc.dram_tensor("cosk", [HD, T], F16, kind="ExternalInput").ap()
    sink_d = nc.dram_tensor("sink", [HD, T], F16, kind="ExternalInput").ap()
    mask_d = nc.dram_tensor("maskm", [HD, 128], F16, kind="ExternalInput").ap()
    outp_d = nc.dram_tensor("outp", [T, D], F16, kind="ExternalOutput").ap()

    mn, mx = mybir.AluOpType.min, mybir.AluOpType.max
    mult, add = mybir.AluOpType.mult, mybir.AluOpType.add
    EXP = mybir.ActivationFunctionType.Exp

    with tile.TileContext(nc) as tc:
        with (
            tc.tile_pool(name="sb", bufs=1) as pool,
            tc.tile_pool(name="ps", bufs=1, space="PSUM") as psum,
        ):
            # persistent tensors
            qkT = pool.tile([128, QKJ, T], F16)       # roped q (scaled) + k
            v_sb = pool.tile([128, TTILES, HD], BF16)  # clipped v, [t%128, t//128, hd]
            # normalized attn^T as fp8 hi planes 0..HPC-1, lo planes HPC..
            attnT = pool.tile([128, 2 * HPC, T], F8)
            cosq = pool.tile([HD, T], F16)
            sinq = pool.tile([HD, T], F16)
            cosk = pool.tile([HD, T], F16)
            sink = pool.tile([HD, T], F16)
            masks = pool.tile([HD, 128], F16)
            # persistent hid slab: per-sweep writes overwrite slices, so the
            # WAR against the previous sweep's readers is tracked per-slice
            # (a per-sweep pool.tile would bump the whole-tile version and
            # serialize the refill behind all of pass B).
            # fp8 hi planes 0..DCH-1, lo planes DCH..2*DCH-1 — every
            # DoubleRow pair is a contiguous [:, a:a+2, :] slice.
            hslab = pool.tile([128, 2 * DCH, 512], F8)

            def load_tables():
                nc.gpsimd.dma_start(cosq[:], cosq_d[:])
                nc.gpsimd.dma_start(sinq[:], sinq_d[:])
                nc.gpsimd.dma_start(cosk[:], cosk_d[:])
                nc.gpsimd.dma_start(sink[:], sink_d[:])
                nc.gpsimd.dma_start(masks[:], mask_d[:])

            def qkv_sweep(tcx, interleave=None):
                # Pass A: q heads j0..j5 accumulate in three 2-bank "wide"
                # PSUM tiles (2 from the sc2 ring + the attnw tile) over all
                # 48 d-chunks; hid lands in a resident SBUF slab. Pass B:
                # k (j6) and v accumulate in the 2-bank ring re-reading the
                # slab (no second hid DMA). During pass B the wides are free
                # again, so the previous chunk's attention chains interleave
                # into the PE stream (hiding the chain latency that
                # otherwise pays off only after the last sweep).
                # All projection matmuls run as fp8(e4m3) DoubleRow 3-term
                # compensation (hi*hi + hi*lo + lo*hi): each DR instruction
                # contracts a contiguous pair of 128-d units at 0.5
                # cycles/row, so a d-pair costs 3 DRs vs 2 fp16 matmuls =
                # 0.75x PE time at ~0.15% error.
                tsl = slice(tcx * 512, (tcx + 1) * 512)
                widesA = [psum.tile([128, 1024], F32, tag="wide", bufs=2,
                                    name=f"qkw{w}") for w in range(2)]
                widesA.append(psum.tile([128, 1024], F32, tag="attnw", bufs=1,
                                        name="qkw2"))
                qk_ps = [widesA[j // 2][:, (j % 2) * 512:(j % 2 + 1) * 512]
                         for j in range(6)]
                for g in range(DG):
                    g4 = slice(g * 512, (g + 1) * 512)
                    wqa = pool.tile([128, 8, 768], F8, tag="wqa", bufs=3)
                    if tcx == 0 and g == 0:
                        for i in range(4):
                            dsl = slice(i * 128, (i + 1) * 128)
                            nc.sync.dma_start(wqa[:, i, :], wqh_d[dsl, 0:768])
                            nc.scalar.dma_start(hslab[:, i, :],
                                                hidh_d[dsl, tsl])
                        for i in range(4):
                            dsl = slice(i * 128, (i + 1) * 128)
                            nc.sync.dma_start(wqa[:, 4 + i, :],
                                              wql_d[dsl, 0:768])
                            nc.scalar.dma_start(hslab[:, DCH + i, :],
                                                hidl_d[dsl, tsl])
                    else:
                        nc.scalar.dma_start(
                            hslab[:, g * 4:(g + 1) * 4, :],
                            hidh_d[g4, tsl].rearrange("(a p) t -> p a t",
                                                      p=128))
                        nc.scalar.dma_start(
                            hslab[:, DCH + g * 4:DCH + (g + 1) * 4, :],
                            hidl_d[g4, tsl].rearrange("(a p) t -> p a t",
                                                      p=128))
                        nc.sync.dma_start(
                            wqa[:, 0:4, :], wqh_d[g4, 0:768].rearrange(
                                "(a p) w -> p a w", p=128))
                        nc.sync.dma_start(
                            wqa[:, 4:8, :], wql_d[g4, 0:768].rearrange(
                                "(a p) w -> p a w", p=128))
                    for i2 in range(2):
                        d = g * 4 + 2 * i2
                        wh = slice(2 * i2, 2 * i2 + 2)
                        wl = slice(4 + 2 * i2, 4 + 2 * i2 + 2)
                        hh = slice(d, d + 2)
                        hl = slice(DCH + d, DCH + d + 2)
                        st, sp = d == 0, d == DCH - 2
                        for j in range(6):
                            jsl = slice(j * 128, (j + 1) * 128)
                            nc.tensor.matmul(qk_ps[j], wqa[:, wh, jsl],
                                             hslab[:, hh, :], start=st,
                                             stop=False, perf_mode=DRM,
                                             skip_group_check=True)
                            nc.tensor.matmul(qk_ps[j], wqa[:, wh, jsl],
                                             hslab[:, hl, :], start=False,
                                             stop=False, perf_mode=DRM,
                                             skip_group_check=True)
                            nc.tensor.matmul(qk_ps[j], wqa[:, wl, jsl],
                                             hslab[:, hh, :], start=False,
                                             stop=sp, perf_mode=DRM,
                                             skip_group_check=True)
                # evac A: clips first (release the wides for the interleaved
                # chains), then ropes for q0..q5 on POOL
                rawsq = []
                for w in range(3):
                    raw2 = pool.tile([128, 1024], F32, tag="raw2", bufs=3,
                                     name=f"raw2_{w}")
                    nc.vector.tensor_scalar(raw2[:], widesA[w][:],
                                            CLIP * A_SCALE, -CLIP * A_SCALE,
                                            mn, mx)
                    rawsq += [raw2[:, 0:512], raw2[:, 512:1024]]

                def rope(j, raw, eng=None):
                    eng = eng or nc.gpsimd
                    xr = pool.tile([128, 512], F32, tag="xr", bufs=3)
                    # SBUF->SBUF partition swap issued from the POOL queue:
                    # keeps the sync queue free so pass B's wqb prefetch
                    # isn't blocked behind 12 swap issues
                    nc.gpsimd.dma_start(xr[0:64, :], raw[64:128, :])
                    nc.gpsimd.dma_start(xr[64:128, :], raw[0:64, :])
                    cosT = cosq if j < HPC else cosk
                    sinT = sinq if j < HPC else sink
                    dst = qkT[:, j, tsl]
                    eng.tensor_tensor(dst, raw, cosT[:, tsl], mult)
                    eng.tensor_tensor(xr[:], xr[:], sinT[:, tsl], mult)
                    eng.tensor_tensor(dst, dst, xr[:], add)

                for j in range(6):
                    rope(j, rawsq[j])
                # pass B: k and v from the slab; previous chunk's chains
                # interleave here
                k_ps = psum.tile([128, 512], F32, tag="bank", bufs=2)
                v_ps = psum.tile([128, 512], F32, tag="bank", bufs=2)
                due = 0.0
                n_y = HPC * ((4 * (tcx - 1) + 4) // 2 + 1) if tcx >= 1 else 0
                rate = n_y / (2 * DG) if interleave is not None else 0.0
                for g in range(DG):
                    g4 = slice(g * 512, (g + 1) * 512)
                    wqb = pool.tile([128, 8, 256], F8, tag="wqb", bufs=2)
                    nc.sync.dma_start(
                        wqb[:, 0:4, :], wqh_d[g4, 768:1024].rearrange(
                            "(a p) w -> p a w", p=128))
                    nc.sync.dma_start(
                        wqb[:, 4:8, :], wql_d[g4, 768:1024].rearrange(
                            "(a p) w -> p a w", p=128))
                    for i2 in range(2):
                        d = g * 4 + 2 * i2
                        wh = slice(2 * i2, 2 * i2 + 2)
                        wl = slice(4 + 2 * i2, 4 + 2 * i2 + 2)
                        hh = slice(d, d + 2)
                        hl = slice(DCH + d, DCH + d + 2)
                        st, sp = d == 0, d == DCH - 2
                        nc.tensor.matmul(k_ps[:], wqb[:, wh, 0:128],
                                         hslab[:, hh, :], start=st,
                                         stop=False, perf_mode=DRM)
                        nc.tensor.matmul(k_ps[:], wqb[:, wh, 0:128],
                                         hslab[:, hl, :], start=False,
                                         stop=False, perf_mode=DRM)
                        nc.tensor.matmul(k_ps[:], wqb[:, wl, 0:128],
                                         hslab[:, hh, :], start=False,
                                         stop=sp, perf_mode=DRM)
                        for s in range(4):
                            # packed quarter-bank outputs: start=True zeroes
                            # the whole 2KB zero-region, so only the first
                            # sub-matmul of the bank may set it
                            s128 = slice(s * 128, (s + 1) * 128)
                            nc.tensor.matmul(v_ps[:, s128],
                                             hslab[:, hh, s128],
                                             wqb[:, wh, 128:256],
                                             start=(st and s == 0),
                                             stop=False, perf_mode=DRM,
                                             skip_group_check=True)
                            nc.tensor.matmul(v_ps[:, s128],
                                             hslab[:, hh, s128],
                                             wqb[:, wl, 128:256],
                                             start=False, stop=False,
                                             perf_mode=DRM,
                                             skip_group_check=True)
                            nc.tensor.matmul(v_ps[:, s128],
                                             hslab[:, hl, s128],
                                             wqb[:, wh, 128:256],
                                             start=False,
                                             stop=(sp and s == 3),
                                             perf_mode=DRM,
                                             skip_group_check=True)
                        due += rate
                        while due >= 1.0:
                            next(interleave, None)
                            due -= 1.0
                if interleave is not None:
                    for _ in interleave:
                        pass
                # evac B
                rawk = pool.tile([128, 512], F32, tag="raw", bufs=2)
                nc.vector.tensor_scalar(rawk[:], k_ps[:], CLIP * A_SCALE,
                                        -CLIP * A_SCALE, mn, mx)
                nc.vector.tensor_scalar(
                    v_sb[:, tcx * 4:(tcx + 1) * 4, :],
                    v_ps[:].rearrange("p (a h) -> p a h", a=4),
                    CLIP * A_SCALE, -CLIP * A_SCALE, mn, mx)
                rope(HPC, rawk[:],
                     eng=nc.vector if tcx == TCH - 1 else None)

            def attn_chain(h, jc):
                # generator: yields once per 2-kt block so the driver can
                # interleave ready out-proj matmuls into the in-order PE
                # stream (fills the PE bubble left by the ACT-paced exp).
                # Scores for a kt-pair land in one 2-bank "wide" PSUM tile so
                # a single 1024-wide exp serves both (less ACT overhead).
                # Diagonal-straddle kt tiles (r = kt-4jc >= 0) compute only
                # the causally-needed q-suffix [128r:512] — 15% less
                # score/v PE work; the in-tile triangle is masked by one
                # [128,128] pattern at the suffix head. The unwritten prefix
                # of those PSUM halves holds stale data; exp covers it but
                # nothing downstream reads it.
                qsl = slice(jc * 512, (jc + 1) * 512)
                n_kt = 4 * jc + 4
                n_b = n_kt // 2
                attnw = psum.tile([128, 1024], F32, tag="attnw", bufs=1)
                attn_ps = attnw[:, 0:512]
                LEAD = 1
                pbs = {}
                # row sums accumulate on DVE in bf16 (2-byte dtype gets the
                # fast DVE mode); suffix-kt adds land in leg 0 (always fully
                # initialized by kt 0), full-width kts alternate legs
                two_legs = jc >= 1
                accs = [pool.tile([128, 512], BF16, tag=f"acc{i}", bufs=1,
                                  name=f"acc{i}")
                        for i in range(2 if two_legs else 1)]
                accs = accs + accs[:1] if not two_legs else accs
                for bstep in range(n_b + LEAD):
                    if bstep < n_b:
                        b = bstep
                        sc2 = psum.tile([128, 1024], F32, tag="wide", bufs=2)
                        for half in range(2):
                            kt = 2 * b + half
                            r = kt - 4 * jc
                            off = 128 * r if r > 0 else 0
                            nc.tensor.matmul(
                                sc2[:, half * 512 + off:(half + 1) * 512],
                                qkT[:, HPC, kt * 128:(kt + 1) * 128],
                                qkT[:, h, jc * 512 + off:(jc + 1) * 512],
                                start=True, stop=True,
                                skip_group_check=True)
                        pb2 = pool.tile([128, 1024], BF16, tag="pb", bufs=3)
                        nc.scalar.activation(pb2[:], sc2[:], EXP)
                        for half in range(2):
                            kt = 2 * b + half
                            r = kt - 4 * jc
                            if r >= 0:
                                msl = slice(half * 512 + 128 * r,
                                            half * 512 + 128 * r + 128)
                                nc.vector.tensor_tensor(
                                    pb2[:, msl], pb2[:, msl], masks[:], mult)
                        for half in range(2):
                            kt = 2 * b + half
                            r = kt - 4 * jc
                            if r > 0:
                                with nc.allow_low_precision(
                                        reason="bf16 row-sum legs"):
                                    nc.vector.tensor_tensor(
                                        accs[0][:, 128 * r:512],
                                        accs[0][:, 128 * r:512],
                                        pb2[:, half * 512 + 128 * r:
                                            (half + 1) * 512], add)
                            else:
                                leg = accs[kt % 2] if two_legs else accs[0]
                                psl = pb2[:, half * 512:(half + 1) * 512]
                                if kt < 2:
                                    nc.vector.tensor_scalar(
                                        leg[:], psl, 0.0, None, add)
                                else:
                                    with nc.allow_low_precision(
                                            reason="bf16 row-sum legs"):
                                        nc.vector.tensor_tensor(
                                            leg[:], leg[:], psl, add)
                        pbs[b] = pb2
                    if bstep >= LEAD:
                        b = bstep - LEAD
                        pb2 = pbs.pop(b)
                        for half in range(2):
                            kt = 2 * b + half
                            r = kt - 4 * jc
                            off = 128 * r if r > 0 else 0
                            st, sp = kt == 0, kt == n_kt - 1
                            nc.tensor.matmul(
                                attnw[:, off:512], v_sb[:, kt, :],
                                pb2[:, half * 512 + off:(half + 1) * 512],
                                start=st, stop=sp, skip_group_check=True)
                    yield
                if two_legs:
                    with nc.allow_low_precision(
                            reason="bf16 row-sum combine, 2e-2 budget"):
                        nc.vector.tensor_tensor(accs[0][:], accs[0][:],
                                                accs[1][:], add)
                # row sums via POOL partition all-reduce (fp32 internal,
                # broadcast to all partitions for free) — no PE rows spent;
                # normalize off the critical path, all-bf16 for fast DVE
                au = pool.tile([128, 512], BF16, tag="au", bufs=4)
                nc.scalar.copy(au[:], attnw[:, 0:512])
                allsum = pool.tile([128, 512], BF16, tag="rec", bufs=4)
                nc.gpsimd.partition_all_reduce(allsum[:], accs[0][:], 128,
                                               bass_isa.ReduceOp.add)
                recb = pool.tile([128, 512], BF16, tag="recb", bufs=4)
                with nc.allow_low_precision(
                        reason="bf16 softmax scale, 2e-2 budget"):
                    nc.vector.reciprocal(recb[:], allsum[:])
                # normalized attn lands as an fp8 hi/lo pair for the
                # DoubleRow out-projection: hi = fp8(a), lo = fp8(a - hi).
                # The multiply runs in place on au (bf16) to save SBUF.
                with nc.allow_low_precision(
                        reason="fp8 hi/lo split, compensated"):
                    nc.vector.tensor_tensor(au[:], au[:], recb[:], mult)
                    nc.scalar.copy(attnT[:, h, qsl], au[:])
                    nc.vector.tensor_tensor(attnT[:, HPC + h, qsl], au[:],
                                            attnT[:, h, qsl],
                                            mybir.AluOpType.subtract)

            def outproj_blocks(jcs):
                # flat generator of out-proj (oc, tt) blocks across t-groups
                # `jcs`; drained one block per chain step so PE never idles
                # while exp paces the chains. The wo weight tile for the
                # first (jc, oc) is DMA'd eagerly at generator creation and
                # each following one is prefetched a step ahead, so no block
                # ever waits on its weight transfer. PSUM->SBUF evacs
                # alternate ACT/DVE (GPSIMD cannot read PSUM).
                # oc-major across the t-groups: one wo load serves every
                # group's blocks for that column chunk (3x less weight DMA
                # on the interleaved portion)
                pairs = [(jc, oc) for oc in range(OCH) for jc in jcs]

                def load_wo(oc):
                    wo = pool.tile([128, 2 * ICH, 512], F8, tag="wo", bufs=3)
                    osl = slice(oc * 512, (oc + 1) * 512)
                    nc.sync.dma_start(wo[:, 0:ICH, :],
                                      wouth_d[:, osl].rearrange(
                                          "(i p) o -> p i o", p=128))
                    nc.sync.dma_start(wo[:, ICH:2 * ICH, :],
                                      woutl_d[:, osl].rearrange(
                                          "(i p) o -> p i o", p=128))
                    return wo

                pending = [load_wo(pairs[0][1])]

                def gen():
                    wo = None
                    last_oc = None
                    for n, (jc, oc) in enumerate(pairs):
                        if oc != last_oc:
                            nxt = next((p[1] for p in pairs[n + 1:]
                                        if p[1] != oc), None)
                            if nxt is not None:
                                pending.append(load_wo(nxt))
                            wo = pending.pop(0)
                            last_oc = oc
                        osl = slice(oc * 512, (oc + 1) * 512)
                        for tt in range(4):
                            t = 4 * jc + tt
                            out_ps = psum.tile([128, 512], F32, tag="bank",
                                               bufs=2)
                            tsl8 = slice(t * 128, (t + 1) * 128)
                            for i2 in range(ICH // 2):
                                i = 2 * i2
                                ah = attnT[:, i:i + 2, tsl8]
                                al = attnT[:, HPC + i:HPC + i + 2, tsl8]
                                wh_ = wo[:, i:i + 2, :]
                                wl_ = wo[:, ICH + i:ICH + i + 2, :]
                                nc.tensor.matmul(out_ps[:], ah, wh_,
                                                 start=(i2 == 0), stop=False,
                                                 perf_mode=DRM)
                                nc.tensor.matmul(out_ps[:], ah, wl_,
                                                 start=False, stop=False,
                                                 perf_mode=DRM)
                                nc.tensor.matmul(out_ps[:], al, wh_,
                                                 start=False,
                                                 stop=(i2 == ICH // 2 - 1),
                                                 perf_mode=DRM)
                            osb = pool.tile([128, 512], F16, tag="osb", bufs=4)
                            th = slice(jc * 512 + tt * 128,
                                       jc * 512 + (tt + 1) * 128)
                            if (oc + tt) % 2 == 0:
                                nc.scalar.copy(osb[:], out_ps[:])
                            else:
                                nc.vector.tensor_copy(osb[:], out_ps[:])
                            nc.sync.dma_start(outp_d[th, osl], osb[:])
                            yield

                return gen()

            # ---- Sweeps with the previous chunk's chains interleaved
            # into pass B; post-QKV: last chunk's chains with all out-proj
            # groups interleaved into the PE stream ----
            def chain_group(jc):
                for h in range(HPC):
                    for _ in attn_chain(h, jc):
                        yield

            load_tables()
            qkv_sweep(0)
            for tcx in range(1, TCH):
                qkv_sweep(tcx, interleave=chain_group(tcx - 1))
            ops = outproj_blocks(list(range(TCH - 1)))
            due = 0.0
            rate = (3 * 4 * OCH) / (HPC * ((4 * 3 + 4) // 2 + 1))
            for h in range(HPC):
                for _ in attn_chain(h, TCH - 1):
                    due += rate
                    while due >= 1.0:
                        next(ops, None)
                        due -= 1.0
            # create the last group's generator before draining the rest so
            # its first weight tile is already in flight
            tail = outproj_blocks([TCH - 1])
            for _ in ops:
                pass
            for _ in tail:
                pass

    nc.compile()
    return nc


def kernel(hidden_states, position_ids, Wqkv, Wout):
    global _compiled
    hidden_states = np.asarray(hidden_states, dtype=np.float32)
    position_ids = np.asarray(position_ids).astype(np.int64)
    Wqkv = np.asarray(Wqkv, dtype=np.float32)
    Wout = np.asarray(Wout, dtype=np.float32)

    if _compiled is None:
        _compiled = _build()
    nc = _compiled

    import ml_dtypes
    E4M3 = ml_dtypes.float8_e4m3

    def split8(x):
        hi = x.astype(E4M3)
        lo = (x - hi.astype(np.float32)).astype(E4M3)
        return np.ascontiguousarray(hi), np.ascontiguousarray(lo)

    # host prep: rope tables (from actual position_ids), masks, shards.
    # Wqkv is pre-scaled by A_SCALE for the fp8 split; the q tables fold
    # score_scale/A_SCALE, the k tables 1/A_SCALE, and the v path's factor
    # is divided out of the final host sum.
    scale = HD ** -0.5
    half = HD // 2
    inv_freq = 1.0 / (THETA ** (np.arange(half, dtype=np.float64) / half))
    freqs = position_ids.astype(np.float64)[None, :] * inv_freq[:, None]  # [64, T]
    cos = np.cos(freqs)
    sin = np.sin(freqs)
    cosf = np.concatenate([cos, cos], 0)
    sinf = np.concatenate([-sin, sin], 0)
    cosq = (cosf * (scale / A_SCALE)).astype(np.float16)
    sinq = (sinf * (scale / A_SCALE)).astype(np.float16)
    cosk = (cosf / A_SCALE).astype(np.float16)
    sink = (sinf / A_SCALE).astype(np.float16)

    p = np.arange(128)[:, None]
    f = np.arange(128)[None, :]
    masks = (f >= p).astype(np.float16)

    hidT = np.ascontiguousarray(hidden_states.T)
    hidh, hidl = split8(hidT)

    q_size = N_HEADS * HD
    in_maps = []
    for c in range(N_CORES):
        qrows = Wqkv[c * HPC * HD:(c + 1) * HPC * HD]
        krows = Wqkv[q_size + c * HD:q_size + (c + 1) * HD]
        vrows = Wqkv[q_size + N_KV * HD + c * HD:q_size + N_KV * HD + (c + 1) * HD]
        wqkvT = np.ascontiguousarray(
            np.concatenate([qrows, krows, vrows], 0).T) * A_SCALE
        wqh, wql = split8(wqkvT)
        woutT = np.ascontiguousarray(
            Wout[:, c * HPC * HD:(c + 1) * HPC * HD].T) * B_SCALE
        wouth, woutl = split8(woutT)
        in_maps.append({
            "hidh": hidh, "hidl": hidl, "wqh": wqh, "wql": wql,
            "wouth": wouth, "woutl": woutl,
            "cosq": cosq, "sinq": sinq, "cosk": cosk, "sink": sink,
            "maskm": masks,
        })

    trace = os.environ.get("DBRX_TRACE", "0") == "1"
    res = run_bass_kernel_spmd(nc, in_maps, core_ids=list(range(N_CORES)),
                               trace=trace)
    kernel.last_result = res

    out = res.results[0]["outp"].astype(np.float32)
    for c in range(1, N_CORES):
        out += res.results[c]["outp"].astype(np.float32)
    # undo the v-path A_SCALE carried through attnT and the Wout B_SCALE
    out /= A_SCALE * B_SCALE
    return out

